# revision 70
# baseline (speedup 1.0000x reference)
"""Chamfer distance kernel for 8 Trainium2 NeuronCores (Bass/Tile).

Problem: pred/target (4, 8192, 3) fp32 -> scalar mean chamfer distance
(bidirectional nearest-neighbor squared distances, mean over batch).

Sharding (data parallel on batch x pred-half): core c handles batch
b = c // 2 and pred-half h = c % 2 (4096 of the 8192 pred points) against
ALL 8192 targets of that batch. Forward mins (over targets) complete per
core; backward row-mins (over preds) are per-half partials that the host
min-combines across the core pair.

Device math per core:
  d2[m, n] = ||q_m||^2 + ||p_n||^2 - 2 q_m . p_n   (m target, n pred)
as ONE K=13 matmul per tile using bf16 hi/lo splitting (~2^-18 relative
error; bf16 streams 1 PE cycle/row where fp32 needs 4):
    Q_aug rows: [qh0..2, qh0..2, ql0..2, q2h, q2l, 1, 1]  (q* = split(-2q))
    P_aug rows: [ph0..2, pl0..2, ph0..2, 1,  1,  p2h, p2l]
  dot = qh.ph + qh.pl + ql.ph + q2 + p2 ~= -2 q.p + ||q||^2 + ||p||^2.

Pipeline per target chunk mi (128 targets on PSUM partitions, all 4096
preds on the free axis, 8 matmuls of [13,128]x[13,512] into two 4-bank
PSUM groups, double buffered). The q-augmentation is negated so the PE
emits -d2 straight into PSUM: every reduction is then a max, which both
DVE tensor_tensor ops (bf16 2x rate) and GPSIMD's
partition_all_reduce(max) — its only ordering op — support. All three
non-PE engines run saturated at ~3.78us/mi:
  - ScalarE (the only engine that can read PSUM at full rate) stages
    each group into one contiguous [128, 4096] bf16 tile. 2 insts/mi,
    3.78us/mi: this paces the whole kernel (~242us over 64 chunks).
  - GPSIMD handles the FORWARD reduce for the first PC=2560 columns:
    one partition_all_reduce(max) per group slice (~3.75us/mi); a DMA
    parks row 0 of each chunk's result in a [64, PC] DRAM collector
    that the host max-folds (a compute-engine write to an arbitrary
    collector partition is illegal — walrus allows partition offsets
    0/32/64/96 only — so the copies ride on the idle DMA engines).
  - VectorE (~3.3us/mi): forward max-accumulate for the remaining
    4096-PC columns in one 2x bf16 op, plus the BACKWARD fold: the two
    n-groups fold into a 512-wide parked tail per mi (all 2x), with
    batched tail reduces every TB=8 chunks; the last 4 chunks skip the
    batch and row-reduce their parked slot directly (tensor_reduce 1x)
    so almost no reduce work trails the final stage.
The host folds collector rows, A-tile partitions and the two core
halves in float64 and applies the reference's maximum(d2, 0) clamp
(it commutes with min), returning the fp32 scalar.

bf16 rounding of d2 before the min reductions adds ~1e-5 relative error
on the reference inputs; the matmul's hi/lo split error is ~2^-18/term.
Op-vocabulary notes for this environment (probed): tensor_tensor_reduce
and tensor_tensor_scan crash at execution under both lowerings;
scalar_tensor_tensor fails walrus' engine check on Pool; GPSIMD cannot
access PSUM; dma_start cannot read PSUM; TRN2 matmul output must be
fp32; only plain tensor_scalar has the 4x_2p DVE mode (TT max/min/add
are 2x_1p; TTR/TensorReduce/Pool/stt/scan are all 1x). Hence the kernel
restricts itself to matmul / ACT copy / DVE TT+TR / PAR / memset / DMA.
"""

import functools

import numpy as np
import ml_dtypes

import concourse.bacc as bacc
import concourse.bass_isa as bass_isa
import concourse.mybir as mybir
import concourse.tile as tile

BF16 = ml_dtypes.bfloat16

B = 4            # batches
N = 8192         # points per cloud
NCORES = 8
NH = N // 2      # preds per core (4096)
K = 13           # augmented contraction dim
MI = N // 128    # 64 target chunks of 128
GF = 2048        # free elements per psum group (4 banks)
NG = NH // GF    # 2 groups per mi
TW = 512         # parked backward-tail width per mi
TB = 16          # mi's per batched tail-reduce pass
BIG = 3.0e38


def _split_bf16(x):
    """fp32 -> (hi, lo) bf16 pair with x ~= hi + lo (error ~2^-18 |x|)."""
    xh = x.astype(BF16)
    xl = (x - xh.astype(np.float32)).astype(BF16)
    return xh, xl


def _aug_inputs(pred, target):
    """Per-core augmented bf16 matrices: {"q_aug": [13, 8192], "p_aug": [13, 4096]}.

    All q_aug rows are NEGATED so the matmul emits -d2 straight into PSUM;
    every downstream reduction is then a max (and raw DMA copies of PSUM are
    already in negated space)."""
    in_maps = []
    for c in range(NCORES):
        b, h = divmod(c, 2)
        q = np.asarray(target[b], dtype=np.float32)              # (8192, 3)
        p = np.asarray(pred[b][h * NH:(h + 1) * NH], dtype=np.float32)

        qh, ql = _split_bf16(2.0 * q)
        q2h, q2l = _split_bf16(-np.sum(q * q, axis=-1, dtype=np.float32))
        onesq = np.full(N, -1.0, dtype=BF16)
        q_aug = np.stack([
            qh[:, 0], qh[:, 1], qh[:, 2],
            qh[:, 0], qh[:, 1], qh[:, 2],
            ql[:, 0], ql[:, 1], ql[:, 2],
            q2h, q2l, onesq, onesq,
        ])                                                       # (13, 8192)

        ph, pl = _split_bf16(p)
        p2h, p2l = _split_bf16(np.sum(p * p, axis=-1, dtype=np.float32))
        onesp = np.ones(NH, dtype=BF16)
        p_aug = np.stack([
            ph[:, 0], ph[:, 1], ph[:, 2],
            pl[:, 0], pl[:, 1], pl[:, 2],
            ph[:, 0], ph[:, 1], ph[:, 2],
            onesp, onesp, p2h, p2l,
        ])                                                       # (13, 4096)
        # single input tensor, columns reordered so mi 0's matmul operands
        # (q cols 0:128 + p group A) are one contiguous leading chunk that
        # a single DMA generation can land first
        qp_aug = np.concatenate([q_aug[:, :128], p_aug[:, :GF],
                                 q_aug[:, 128:], p_aug[:, GF:]], axis=1)
        in_maps.append({"qp_aug": np.ascontiguousarray(qp_aug)})
    return in_maps


def _qcol(i):
    """qp_aug column of q_aug column i."""
    return i if i < 128 else 2048 + i


def _pcol(j):
    """qp_aug column of p_aug column j."""
    return 128 + j if j < GF else N + j


ST = 4      # final chunks using within-group backward pairing
F0W = 2048  # mi-0 stage sliver width (fill vs ACT-init tradeoff: full-width
            # wins, the extra per-sliver ACT inits cost more than the
            # earlier start buys)
PC = 2560   # columns (preds) whose forward reduce runs on the Pool engine
DS = 0      # columns of group B staged by the DVE instead of the ACT
DM = 0      # columns of group A staged as fp32 by the (idle) DMA engines


@functools.lru_cache(maxsize=8)
def _build_program(mi_count=MI, mode="bf16fold",
                   tail_sched=(8,) * 7 + (4,),
                   ship_a_full=True, split_dma=True, warmup=False, fine0=True,
                   pool_cols=PC, dve_stage=DS, dma_stage=DM, bwd="fold"):
    """mode "alldve": fp32 reduce + min-accumulate straight from PSUM (no
    bf16 rounding, ~2x slower). mode "bf16fold": the pipeline described in
    the module docstring, plus a Pool-engine forward path: per mi the GPSIMD
    partition_all_reduce(max) collapses the first `pool_cols` staged columns
    to their per-chunk column max and a DMA parks row 0 in a [64, pool_cols]
    DRAM collector (the host folds the 64 rows); the DVE forward TT then only
    covers the remaining 4096-pool_cols columns."""
    nc = bacc.Bacc("TRN2", target_bir_lowering=False, debug=False,
                   num_devices=NCORES)
    f32 = mybir.dt.float32
    bf16 = mybir.dt.bfloat16
    mn = mybir.AluOpType.min
    mx = mybir.AluOpType.max

    a_dt = f32 if mode == "alldve" else bf16
    r_cols = NG * MI if mode == "alldve" else MI
    if mode == "alldve":
        pool_cols = 0
    dve_cols = NH - pool_cols

    qp_dram = nc.dram_tensor("qp_aug", [K, N + NH], bf16, kind="ExternalInput")
    a_rows = 128 if (ship_a_full or mode == "alldve") else 1
    a_dram = nc.dram_tensor("a_out", [a_rows, dve_cols],
                            a_dt if (ship_a_full or mode == "alldve") else f32,
                            kind="ExternalOutput")
    r_dram = nc.dram_tensor("r_out", [128, r_cols], f32, kind="ExternalOutput")
    assert dma_stage in (0, 512)
    assert pool_cols >= dma_stage
    if pool_cols > dma_stage:
        c_dram = nc.dram_tensor("c_out", [MI, pool_cols - dma_stage], bf16,
                                kind="ExternalOutput")
    if dma_stage:
        c32_dram = nc.dram_tensor("c32_out", [MI, dma_stage], f32,
                                  kind="ExternalOutput")
    last_fwd_ship = mode == "bf16fold" and ship_a_full and mi_count == MI
    if last_fwd_ship:
        # the last chunk skips the forward accumulate; its staged tile ships
        # raw and the host folds it, so the a_out DMA leaves after chunk 62
        a2_dram = nc.dram_tensor("a2_out", [128, dve_cols], bf16,
                                 kind="ExternalOutput")

    with tile.TileContext(nc) as tc:
        with tc.tile_pool(name="const", bufs=1) as cpool, \
             tc.tile_pool(name="stage", bufs=5) as spool, \
             tc.tile_pool(name="fold", bufs=2) as fpool, \
             tc.tile_pool(name="par", bufs=3) as parpool, \
             tc.tile_pool(name="psum", bufs=2, space="PSUM") as ppool:
            qp_sb = cpool.tile([K, N + NH], bf16)
            a_sb = cpool.tile([128, dve_cols], a_dt)
            r_sb = cpool.tile([128, r_cols], f32)
            rt_sb = cpool.tile([128, MI * TW], bf16)

            if split_dma:
                # input DMAs in consumption order: mi 0's chunk (q 0:128 +
                # p group A, contiguous in the qp layout) in ONE generation,
                # then p group B (mi 0's second half), then q for the first
                # few mi's, then the bulk
                c1 = 128 + GF
                nc.sync.dma_start(out=qp_sb[:, :c1], in_=qp_dram.ap()[:, :c1])
                nc.sync.dma_start(out=qp_sb[:, c1:N + GF],
                                  in_=qp_dram.ap()[:, c1:N + GF])
                nc.sync.dma_start(out=qp_sb[:, N + GF:],
                                  in_=qp_dram.ap()[:, N + GF:])
            else:
                nc.sync.dma_start(out=qp_sb[:], in_=qp_dram.ap())
            nc.gpsimd.memset(a_sb[:], BIG if mode == "alldve" else -BIG)

            warm_sb = None
            if warmup:
                # PE p-state warmup source while input DMAs are in flight;
                # dummy matmuls go into mi 0's own PSUM tile (emitted below)
                # so no extra PSUM ring slot is consumed
                warm_sb = cpool.tile([K, 128], bf16)
                nc.gpsimd.memset(warm_sb[:], 0.0)

            # backward-tail batch endpoints: {last-mi-of-batch: batch size}.
            # The final batches shrink so less reduce work trails the last
            # matmul/stage step.
            tail_ends = {}
            acc = 0
            for sz in tail_sched:
                acc += sz
                tail_ends[acc - 1] = sz
            assert acc <= MI  # mi's beyond acc row-reduce directly

            r_shipped = 0
            split_tail = (ST if (mode == "bf16fold" and bwd == "fold"
                                and not dma_stage) else 0)
            for mi in range(mi_count):
                s_sb = None
                s_par = None
                s32 = None
                fold_pre = None
                for g in range(NG):
                    ps = ppool.tile([128, GF], f32, tag="ps")
                    # finer matmuls for mi 0 so the PE p-state ramp (and the
                    # first stage slivers) start ~3us earlier
                    mw = (128 if g == 0 else 256) if (mi == 0 and fine0) \
                        else 512
                    if mi == 0 and g == 0 and warm_sb is not None:
                        # tiny dummy matmuls bridge the PE p-state ramp
                        # through the input-DMA wait; their [0:64] output is
                        # overwritten by the first real matmul (start=True)
                        for _ in range(24):
                            nc.tensor.matmul(
                                ps[:, 0:64], warm_sb[:, :128],
                                warm_sb[:, :64], start=True, stop=True)
                    qc = _qcol(mi * 128)
                    for j in range(GF // mw):
                        c0 = _pcol(g * GF + j * mw)
                        nc.tensor.matmul(
                            ps[:, j * mw:(j + 1) * mw],
                            qp_sb[:, qc:qc + 128],
                            qp_sb[:, c0:c0 + mw],
                            start=True, stop=True,
                        )
                    if mode == "alldve":
                        asl = a_sb[:, g * GF:(g + 1) * GF]
                        nc.vector.tensor_reduce(
                            out=r_sb[:, g * MI + mi: g * MI + mi + 1],
                            in_=ps[:], axis=mybir.AxisListType.X, op=mn)
                        nc.vector.tensor_tensor(
                            out=asl, in0=ps[:], in1=asl, op=mn)
                    elif mode == "bf16fold":
                        # stage as NEGATED bf16 (scale=-1 is free in the
                        # ACT copy); all reductions below become max, so
                        # partition_all_reduce(max) applies directly.
                        if s_sb is None:
                            s_sb = spool.tile([128, NG * GF], bf16,
                                              tag="stage")
                        dm = dma_stage if g == 0 else 0
                        if dm:
                            # idle-DMA staging: raw fp32 copy of the first
                            # dma_stage psA columns (already -d2)
                            s32 = spool.tile([128, dma_stage], f32,
                                             tag="s32")
                            nc.sync.dma_start(out=s32[:], in_=ps[:, :dm])
                        if mi == 0 and fine0:
                            # fill acceleration: stage + fwd-accumulate mi 0
                            # in slivers right behind each matmul so the ACT
                            # chain starts as early as possible
                            off = dm
                            while off < GF:
                                w = F0W
                                lo = g * GF + off
                                nc.scalar.mul(s_sb[:, lo:lo + w],
                                              ps[:, off:off + w], 1.0)
                                dlo = max(lo, pool_cols)
                                if dlo < lo + w:
                                    nc.vector.tensor_tensor(
                                        out=a_sb[:, dlo - pool_cols:
                                                 lo + w - pool_cols],
                                        in0=s_sb[:, dlo:lo + w],
                                        in1=a_sb[:, dlo - pool_cols:
                                                 lo + w - pool_cols], op=mx)
                                off += w
                        else:
                            lo = g * GF
                            if g == NG - 1 and dve_stage:
                                # DVE takes the tail slice of group B's stage
                                # (1x fp32-from-PSUM tensor_scalar) to unload
                                # ACT; psB is reused late enough that the
                                # extra PSUM reader doesn't stall the ping-pong
                                nc.scalar.mul(
                                    s_sb[:, lo:lo + GF - dve_stage],
                                    ps[:, :GF - dve_stage], 1.0)
                                nc.vector.tensor_scalar(
                                    out=s_sb[:, lo + GF - dve_stage:lo + GF],
                                    in0=ps[:, GF - dve_stage:], scalar1=1.0,
                                    scalar2=None, op0=mybir.AluOpType.mult)
                            else:
                                nc.scalar.mul(s_sb[:, lo + dm:lo + GF],
                                              ps[:, dm:], 1.0)
                        if dm:
                            # Pool forward for the fp32-staged columns
                            par32 = parpool.tile([128, dma_stage], f32,
                                                 tag="par32")
                            nc.gpsimd.partition_all_reduce(
                                par32[:], s32[:], 128, bass_isa.ReduceOp.max)
                            nc.sync.dma_start(
                                out=c32_dram.ap()[mi:mi + 1, :],
                                in_=par32[0:1, :])
                        if mi >= mi_count - split_tail and g == 0:
                            # drain shaping: the final chunks pair backward
                            # columns WITHIN each group, so group A's half of
                            # the fold runs as soon as group A is staged
                            # instead of waiting for the full stage
                            fold_pre = fpool.tile([128, GF], bf16, tag="fold")
                            nc.vector.tensor_tensor(
                                out=fold_pre[:, :GF // 2],
                                in0=s_sb[:, :GF // 2],
                                in1=s_sb[:, GF // 2:GF], op=mx)
                            nc.vector.tensor_tensor(
                                out=fold_pre[:, :GF // 4],
                                in0=fold_pre[:, :GF // 4],
                                in1=fold_pre[:, GF // 4:GF // 2], op=mx)
                        if (last_fwd_ship and mi == mi_count - 1
                                and g == NG - 1):
                            # drain shaping: ship the raw staged tile (host
                            # folds it) and the accumulator BEFORE the last
                            # PAR's collector DMA so the final r DMA's
                            # generation isn't queued behind the big a2
                            # transfer
                            nc.sync.dma_start(out=a2_dram.ap(),
                                              in_=s_sb[:, pool_cols:])
                            nc.sync.dma_start(out=a_dram.ap(), in_=a_sb[:])
                        if not dma_stage:
                            # Pool forward: per-group column max of this
                            # group's pool columns, row 0 parked in the DRAM
                            # collector for the host fold.
                            gc0, gc1 = g * GF, min((g + 1) * GF, pool_cols)
                            if gc0 < gc1:
                                if s_par is None:
                                    s_par = parpool.tile(
                                        [128, pool_cols], bf16, tag="par")
                                nc.gpsimd.partition_all_reduce(
                                    s_par[:, gc0:gc1], s_sb[:, gc0:gc1], 128,
                                    bass_isa.ReduceOp.max)
                                nc.sync.dma_start(
                                    out=c_dram.ap()[mi:mi + 1, gc0:gc1],
                                    in_=s_par[0:1, gc0:gc1])
                    else:
                        raise ValueError(mode)
                if mode == "bf16fold" and dma_stage and pool_cols > dma_stage:
                    # Pool forward for the bf16-staged pool columns, one PAR
                    # spanning both groups' staged regions
                    s_par = parpool.tile([128, pool_cols - dma_stage], bf16,
                                         tag="par")
                    nc.gpsimd.partition_all_reduce(
                        s_par[:], s_sb[:, dma_stage:pool_cols], 128,
                        bass_isa.ReduceOp.max)
                    nc.sync.dma_start(
                        out=c_dram.ap()[mi:mi + 1, :], in_=s_par[0:1, :])
                if mode == "bf16fold":
                    if last_fwd_ship and mi == mi_count - 1:
                        pass  # a2/a shipped inside the g-loop
                    elif not (mi == 0 and fine0):
                        # forward max-accumulate (DVE columns only), one 2x
                        # bf16 op
                        nc.vector.tensor_tensor(
                            out=a_sb[:], in0=s_sb[:, pool_cols:],
                            in1=a_sb[:], op=mx)
                    if bwd == "ttr":
                        # backward: one 1x TTR folds the two n-groups and
                        # row-reduces the fold in the same pass; the folded
                        # tile itself goes to a scratch buffer.
                        fold = fpool.tile([128, GF], bf16, tag="fold")
                        nc.vector.tensor_tensor_reduce(
                            out=fold[:], in0=s_sb[:, :GF], in1=s_sb[:, GF:],
                            scale=1.0, scalar=-BIG, op0=mx, op1=mx,
                            accum_out=r_sb[:, mi:mi + 1])
                        if mi % TB == TB - 1 or mi == mi_count - 1:
                            # ship finished r columns during the loop so the
                            # final drain only flushes the last chunk
                            c0 = (mi // TB) * TB
                            nc.sync.dma_start(
                                out=r_dram.ap()[:, c0:mi + 1],
                                in_=r_sb[:, c0:mi + 1])
                    else:
                        # backward: fold the staged columns (valid under the
                        # row-reduce), max-halve at 2x down to a 512-wide
                        # parked tail in this mi's R-slot; tails reduce in
                        # batched 2x passes every TB mi's (cheaper than the
                        # per-mi 1x tensor_reduce).
                        slot = rt_sb[:, mi * TW:(mi + 1) * TW]
                        if fold_pre is not None:
                            # group B's half of the within-group fold, then
                            # merge with group A's prefolded half
                            nc.vector.tensor_tensor(
                                out=fold_pre[:, GF // 2:GF],
                                in0=s_sb[:, GF:GF + GF // 2],
                                in1=s_sb[:, GF + GF // 2:], op=mx)
                            nc.vector.tensor_tensor(
                                out=fold_pre[:, GF // 2:GF // 2 + GF // 4],
                                in0=fold_pre[:, GF // 2:GF // 2 + GF // 4],
                                in1=fold_pre[:, GF // 2 + GF // 4:GF], op=mx)
                            nc.vector.tensor_tensor(
                                out=slot, in0=fold_pre[:, :TW],
                                in1=fold_pre[:, GF // 2:GF // 2 + TW], op=mx)
                            assert mi >= sum(tail_sched)
                            nc.vector.tensor_reduce(
                                out=r_sb[:, mi:mi + 1], in_=slot,
                                axis=mybir.AxisListType.X, op=mx)
                            continue
                        fold = fpool.tile([128, GF], bf16, tag="fold")
                        if dma_stage:
                            # bf16 region [dm:4096] = 3584 cols: 1792 -> 896
                            # -> 448 into the slot head; fp32 region s32
                            # (512): 256 (1x) -> 128 -> 64 into the slot tail
                            dh = (NG * GF - dma_stage) // 2      # 1792
                            nc.vector.tensor_tensor(
                                out=fold[:, :dh],
                                in0=s_sb[:, dma_stage:dma_stage + dh],
                                in1=s_sb[:, dma_stage + dh:], op=mx)
                            nc.vector.tensor_tensor(
                                out=fold[:, :dh // 2], in0=fold[:, :dh // 2],
                                in1=fold[:, dh // 2:dh], op=mx)
                            mq = dma_stage // 2                  # 256
                            nc.vector.tensor_tensor(
                                out=fold[:, dh:dh + mq], in0=s32[:, :mq],
                                in1=s32[:, mq:], op=mx)
                            nc.vector.tensor_tensor(
                                out=fold[:, dh:dh + mq // 2],
                                in0=fold[:, dh:dh + mq // 2],
                                in1=fold[:, dh + mq // 2:dh + mq], op=mx)
                            nc.vector.tensor_tensor(
                                out=slot[:, :dh // 4], in0=fold[:, :dh // 4],
                                in1=fold[:, dh // 4:dh // 2], op=mx)
                            nc.vector.tensor_tensor(
                                out=slot[:, dh // 4:TW],
                                in0=fold[:, dh:dh + mq // 4],
                                in1=fold[:, dh + mq // 4:dh + mq // 2], op=mx)
                        else:
                            nc.vector.tensor_tensor(
                                out=fold[:], in0=s_sb[:, :GF],
                                in1=s_sb[:, GF:], op=mx)
                            nc.vector.tensor_tensor(
                                out=fold[:, :GF // 2], in0=fold[:, :GF // 2],
                                in1=fold[:, GF // 2:], op=mx)
                            nc.vector.tensor_tensor(
                                out=slot, in0=fold[:, :TW],
                                in1=fold[:, TW:2 * TW], op=mx)
                        if mi >= sum(tail_sched):
                            # drain shaping: the last mi's skip the batched
                            # tail and row-reduce their parked slot directly
                            nc.vector.tensor_reduce(
                                out=r_sb[:, mi:mi + 1], in_=slot,
                                axis=mybir.AxisListType.X, op=mx)
                            continue
                        if mi in tail_ends:
                            nb = tail_ends[mi]
                            c0 = (mi - nb + 1) * TW
                            v = rt_sb[:, c0:c0 + nb * TW].rearrange(
                                "p (m w) -> p m w", m=nb)
                            w = TW // 2
                            while w >= 2:
                                nc.vector.tensor_tensor(
                                    out=v[:, :, :w], in0=v[:, :, :w],
                                    in1=v[:, :, w:2 * w], op=mx)
                                w //= 2
                            nc.vector.tensor_tensor(
                                out=r_sb[:, mi - nb + 1: mi + 1],
                                in0=v[:, :, 0], in1=v[:, :, 1], op=mx)
                            # ship finished r columns during the loop so the
                            # final drain only flushes the last chunk (late
                            # batches coalesce into one end-of-loop DMA)
                            if mi < mi_count - 8:
                                nc.sync.dma_start(
                                    out=r_dram.ap()[:, r_shipped:mi + 1],
                                    in_=r_sb[:, r_shipped:mi + 1])
                                r_shipped = mi + 1

            if last_fwd_ship:
                pass  # a_out already shipped inside the loop
            elif ship_a_full or mode == "alldve":
                # full [128, dve_cols] accumulator; host takes the partition min
                nc.sync.dma_start(out=a_dram.ap(), in_=a_sb[:])
            else:
                # device-side partition max (negated space), ship one row
                pm_sb = cpool.tile([128, dve_cols], f32)
                for hh in range(2):
                    sl = slice(hh * dve_cols // 2, (hh + 1) * dve_cols // 2)
                    nc.gpsimd.partition_all_reduce(
                        pm_sb[:, sl], a_sb[:, sl], 128,
                        bass_isa.ReduceOp.max)
                nc.sync.dma_start(out=a_dram.ap(), in_=pm_sb[0:1, :])
            if mode != "bf16fold":
                nc.sync.dma_start(out=r_dram.ap(), in_=r_sb[:])
            elif r_shipped < mi_count:
                nc.sync.dma_start(out=r_dram.ap()[:, r_shipped:mi_count],
                                  in_=r_sb[:, r_shipped:mi_count])

    nc.compile()
    return nc


# ---------------------------------------------------------------------------
# Execution: a cached jitted shard_map runner over the 8 axon devices
# (rebuilding it per call would re-trace and cost ~0.5s/call), with a
# fallback to the stock run_bass_kernel_spmd path.
# ---------------------------------------------------------------------------

_RUNNER_CACHE = {}


def _make_runner(nc):
    import jax
    from jax.sharding import Mesh, PartitionSpec
    from jax.experimental.shard_map import shard_map
    from concourse import bass2jax
    from concourse.bass2jax import _bass_exec_p, install_neuronx_cc_hook

    install_neuronx_cc_hook()
    partition_name = nc.partition_id_tensor.name if nc.partition_id_tensor else None
    in_names, out_names, out_avals, zero_shapes = [], [], [], []
    for alloc in nc.m.functions[0].allocations:
        if not isinstance(alloc, mybir.MemoryLocationSet):
            continue
        name = alloc.memorylocations[0].name
        if alloc.kind == "ExternalInput":
            if name != partition_name:
                in_names.append(name)
        elif alloc.kind == "ExternalOutput":
            np_dtype = mybir.dt.np(alloc.dtype)
            shape = tuple(alloc.tensor_shape)
            out_names.append(name)
            out_avals.append(jax.core.ShapedArray(shape, np_dtype))
            zero_shapes.append((shape, np_dtype))

    n_params, n_outs = len(in_names), len(out_avals)
    all_in_names = list(in_names) + list(out_names)
    if partition_name is not None:
        all_in_names.append(partition_name)
    donate = tuple(range(n_params, n_params + n_outs))

    def _body(*args):
        operands = list(args)
        if partition_name is not None:
            operands.append(bass2jax.partition_id_tensor())
        outs = _bass_exec_p.bind(
            *operands, out_avals=tuple(out_avals),
            in_names=tuple(all_in_names), out_names=tuple(out_names),
            lowering_input_output_aliases=(),
            sim_require_finite=True, sim_require_nnan=True, nc=nc)
        return tuple(outs)

    devices = jax.devices()[:NCORES]
    mesh = Mesh(np.asarray(devices), ("core",))
    del donate  # outputs are fully written by the kernel; skip donation so
    # the zero "output seed" buffers can stay resident on device across calls
    sharded = jax.jit(
        shard_map(_body, mesh=mesh,
                  in_specs=(PartitionSpec("core"),) * (n_params + n_outs),
                  out_specs=(PartitionSpec("core"),) * n_outs,
                  check_rep=False),
        keep_unused=True)
    from jax.sharding import NamedSharding
    sh = NamedSharding(mesh, PartitionSpec("core"))
    zeros_dev = [
        jax.device_put(np.zeros((NCORES * s[0], *s[1:]), d), sh)
        for s, d in zero_shapes]

    def run(in_maps):
        concat_in = [
            np.concatenate([np.asarray(in_maps[c][name])
                            for c in range(NCORES)], axis=0)
            for name in in_names]
        outs = sharded(*concat_in, *zeros_dev)
        return [
            {name: np.asarray(outs[i]).reshape(NCORES, *out_avals[i].shape)[c]
             for i, name in enumerate(out_names)}
            for c in range(NCORES)]

    return run


def _run_spmd(nc, in_maps):
    key = id(nc)
    try:
        if key not in _RUNNER_CACHE:
            _RUNNER_CACHE[key] = _make_runner(nc)
        return _RUNNER_CACHE[key](in_maps)
    except Exception:
        from concourse.bass_utils import run_bass_kernel_spmd
        return run_bass_kernel_spmd(
            nc, in_maps, core_ids=list(range(NCORES))).results


def _host_reduce(results):
    """Combine per-core outputs into the final scalar (float64 internally)."""
    chamfers = []
    for b in range(B):
        fs = []
        bvecs = []
        for h in range(2):
            res = results[2 * b + h]
            A0 = np.asarray(res["a_out"])                     # [128, dve_cols]
            A = A0.astype(np.float64)
            R = np.asarray(res["r_out"]).astype(np.float64)   # [128, 64|128]
            if A0.dtype == BF16:                              # negated space
                Am = A.max(axis=0)
                if "a2_out" in res:                           # unaccumulated
                    A2 = np.asarray(res["a2_out"]).astype(np.float64)
                    Am = np.maximum(Am, A2.max(axis=0))       # last chunk
                fdve = -Am                                    # fmin = -max(-d2)
                parts = []
                if "c32_out" in res:                          # fp32 pool cols
                    C32 = np.asarray(res["c32_out"]).astype(np.float64)
                    parts.append(-C32.max(axis=0))
                if "c_out" in res:                            # bf16 pool cols
                    C = np.asarray(res["c_out"]).astype(np.float64)
                    parts.append(-C.max(axis=0))
                parts.append(fdve)
                fs.append(np.concatenate(parts))
            else:
                fs.append(A.min(axis=0))                      # [4096]
            if R.shape[1] == 2 * MI:                          # alldve layout
                R = np.minimum(R[:, :MI], R[:, MI:])          # [128, 64]
            else:
                R = -R                                        # negated space
            bvecs.append(R.T.reshape(N))                      # m = 128*mi + p
        f = np.maximum(np.concatenate(fs), 0.0)               # [8192] fwd mins
        bv = np.maximum(np.minimum(bvecs[0], bvecs[1]), 0.0)  # [8192] bwd mins
        chamfers.append(f.mean() + bv.mean())
    return np.float32(np.mean(chamfers))


def kernel(pred, target):
    pred = np.asarray(pred, dtype=np.float32)
    target = np.asarray(target, dtype=np.float32)
    in_maps = _aug_inputs(pred, target)
    nc = _build_program()
    results = _run_spmd(nc, in_maps)
    return np.array(_host_reduce(results), dtype=np.float32)

